# revision 7
# baseline (speedup 1.0000x reference)
"""BinaryNADE Trainium2 kernel (8-core SPMD, h-sharded).

Math (reference):
    base = c + W_ctx @ context                      # [H]
    contrib = W_sol * s[None, :]                    # [H, D]
    A = base[:, None] + exclusive_cumsum_d(contrib) # [H, D]
    Hmat = sigmoid(A)                               # [H, D]
    logit[d] = b[d] + sum_h U[d, h] * Hmat[h, d]
    p_dist = sigmoid(logit)
    p_val = prod(p_dist**s + (1 - p_dist[0])**(1 - s))

Sharding: each of the 8 cores owns 512 rows of W (and the matching 512
columns of U). The exclusive cumsum runs along the free axis per h-row, so
h-sharding makes every core fully independent; the only cross-core step is
the final sum of per-core partial dot products, done on the host along with
the (tiny) final sigmoid / p_val reduction.

Per-core device pipeline, d in chunks of 2048:
    DMA ws chunk            (sync/HWDGE)
    contrib = ws * s_bcast  (GPSIMD tensor_mul)
    A = scan(contrib)       (VectorE tensor_tensor_scan, chained via initial=)
    Hm = sigmoid(A + base)  (ScalarE activation, per-partition bias)
    V = Hm * utT chunk      (VectorE tensor_mul)
    psum[blk, :] += ones^T V (TensorE matmuls with one-hot-column lhsT)
The exclusive shift is baked in on the host: ws and s are passed with a zero
column prepended, so chunk k reads the (d-1)-shifted window uniformly.
"""

import numpy as np

import concourse.bass as bass
import concourse.bacc as bacc
import concourse.mybir as mybir
from concourse.tile import TileContext
from concourse.bass_utils import run_bass_kernel_spmd

F32 = mybir.dt.float32

TRACE = False       # set by test harness to capture an NTFF profile
LAST_RESULT = None

DIM_SOL = 8192      # D
DIM_CONTEXT = 2048  # C
DIM_HIDDEN = 4096   # H
N_CORES = 8
H_SH = DIM_HIDDEN // N_CORES   # 512 hidden rows per core


def build_core_kernel(h_sh=H_SH, c_dim=DIM_CONTEXT, d_dim=DIM_SOL, dc=2048):
    """Build the per-core Bass program. All cores run this same program on
    their own shard (SPMD via run_bass_kernel_spmd in_maps)."""
    assert h_sh % 128 == 0 and c_dim % 128 == 0 and d_dim % dc == 0 and dc % 512 == 0
    ht = h_sh // 128          # h-tiles per core
    nk = d_dim // dc          # d chunks
    nb = dc // 512            # 512-blocks per chunk
    nr = d_dim // 512         # psum rows (total 512-blocks)
    assert nr <= 128
    kt_n = c_dim // 128 + 1   # wct k-tiles, incl. the c-vector row
    cp = kt_n * 128

    nc = bacc.Bacc("TRN2", target_bir_lowering=False, debug=False)

    wct = nc.dram_tensor("wct", [cp, h_sh], F32, kind="ExternalInput")
    ctxc = nc.dram_tensor("ctxc", [128, kt_n], F32, kind="ExternalInput")
    ws = nc.dram_tensor("ws", [h_sh, d_dim + 1], F32, kind="ExternalInput")
    ut = nc.dram_tensor("ut", [h_sh, d_dim], F32, kind="ExternalInput")
    sbt = nc.dram_tensor("sbt", [128, d_dim + 1], F32, kind="ExternalInput")
    emat = nc.dram_tensor("emat", [128, nr * nr], F32, kind="ExternalInput")
    outp = nc.dram_tensor("outp", [nr, 512], F32, kind="ExternalOutput")

    with TileContext(nc) as tc:
        with (
            tc.tile_pool(name="const", bufs=1) as constp,
            tc.tile_pool(name="wctp", bufs=2) as wctp,
            tc.tile_pool(name="wsp", bufs=3) as wsp,
            tc.tile_pool(name="utp", bufs=3) as utp,
            tc.tile_pool(name="ctrp", bufs=2) as ctrp,
            tc.tile_pool(name="apool", bufs=3) as apool,
            tc.tile_pool(name="hpool", bufs=2) as hpool,
            tc.tile_pool(name="vpool", bufs=2) as vpool,
            tc.tile_pool(name="psout", bufs=1, space="PSUM") as psoutp,
            tc.tile_pool(name="psbase", bufs=1, space="PSUM") as psbasep,
        ):
            sb_sb = constp.tile([128, d_dim + 1], F32)
            nc.sync.dma_start(out=sb_sb[:, :], in_=sbt[:, :])
            ctx_sb = constp.tile([128, kt_n], F32)
            nc.sync.dma_start(out=ctx_sb[:, :], in_=ctxc[:, :])
            em_sb = constp.tile([128, nr * nr], F32)
            nc.sync.dma_start(out=em_sb[:, :], in_=emat[:, :])
            base_col = constp.tile([128, ht], F32)

            # base[h] = sum_c W_ctx[h, c] * ctx[c]  (+ c[h] via the augmented
            # row at c_dim with ctx value 1.0), one psum column per h-tile.
            base_ps = [psbasep.tile([128, 1], F32, name=f"base_ps{t}",
                                    tag=f"base_ps{t}")
                       for t in range(ht)]
            for kt in range(kt_n):
                w_t = wctp.tile([128, h_sh], F32)
                nc.sync.dma_start(out=w_t[:, :], in_=wct[128 * kt:128 * (kt + 1), :])
                for t in range(ht):
                    nc.tensor.matmul(
                        base_ps[t][:, :],
                        w_t[:, 128 * t:128 * (t + 1)],
                        ctx_sb[:, kt:kt + 1],
                        start=(kt == 0),
                        stop=(kt == kt_n - 1),
                    )
            for t in range(ht):
                nc.scalar.copy(base_col[:, t:t + 1], base_ps[t][:, :])

            ps_out = psoutp.tile([nr, 512], F32)
            first_mm = True
            for t in range(ht):
                a_prev = None
                for k in range(nk):
                    ws_t = wsp.tile([128, dc], F32)
                    nc.sync.dma_start(
                        out=ws_t[:, :],
                        in_=ws[128 * t:128 * (t + 1), dc * k:dc * k + dc],
                    )
                    ut_t = utp.tile([128, dc], F32)
                    nc.sync.dma_start(
                        out=ut_t[:, :],
                        in_=ut[128 * t:128 * (t + 1), dc * k:dc * k + dc],
                    )
                    ctr = ctrp.tile([128, dc], F32)
                    nc.gpsimd.tensor_mul(
                        ctr[:, :], ws_t[:, :], sb_sb[:, dc * k:dc * k + dc]
                    )
                    a_t = apool.tile([128, dc], F32)
                    init = 0.0 if k == 0 else a_prev[:, dc - 1:dc]
                    nc.vector.tensor_tensor_scan(
                        a_t[:, :], ctr[:, :], ctr[:, :], init,
                        mybir.AluOpType.add, mybir.AluOpType.bypass,
                    )
                    h_t = hpool.tile([128, dc], F32)
                    nc.scalar.activation(
                        h_t[:, :], a_t[:, :],
                        mybir.ActivationFunctionType.Sigmoid,
                        bias=base_col[:, t:t + 1], scale=1.0,
                    )
                    v_t = vpool.tile([128, dc], F32)
                    nc.vector.tensor_mul(v_t[:, :], h_t[:, :], ut_t[:, :])
                    for b2 in range(nb):
                        cblk = k * nb + b2
                        nc.tensor.matmul(
                            ps_out[:, :],
                            em_sb[:, nr * cblk:nr * (cblk + 1)],
                            v_t[:, 512 * b2:512 * (b2 + 1)],
                            start=first_mm,
                            stop=(t == ht - 1 and cblk == nr - 1),
                        )
                        first_mm = False
                    a_prev = a_t

            out_sb = constp.tile([nr, 512], F32)
            nc.scalar.copy(out_sb[:, :], ps_out[:, :])
            nc.sync.dma_start(out=outp[:, :], in_=out_sb[:, :])

    nc.compile()
    return nc


def make_in_maps(context, solution, W, U, c,
                 h_sh=H_SH, c_dim=DIM_CONTEXT, d_dim=DIM_SOL, n_cores=N_CORES):
    """Host-side shard/layout prep. Layout only — no model math happens here."""
    f = np.float32
    kt_n = c_dim // 128 + 1
    cp = kt_n * 128
    nr = d_dim // 512

    ctxa = np.zeros(cp, f)
    ctxa[:c_dim] = context
    ctxa[c_dim] = 1.0
    ctxc = np.ascontiguousarray(ctxa.reshape(kt_n, 128).T)

    sb = np.zeros((1, d_dim + 1), f)
    sb[0, 1:] = solution
    sbt = np.ascontiguousarray(np.broadcast_to(sb, (128, d_dim + 1)))

    emat = np.zeros((128, nr * nr), f)
    for cc in range(nr):
        emat[:, nr * cc + cc] = 1.0

    in_maps = []
    for core in range(n_cores):
        h0 = core * h_sh
        wct = np.zeros((cp, h_sh), f)
        wct[:c_dim] = W[h0:h0 + h_sh, :c_dim].T
        wct[c_dim] = c[h0:h0 + h_sh]
        ws = np.zeros((h_sh, d_dim + 1), f)
        ws[:, 1:] = W[h0:h0 + h_sh, c_dim:]
        ut = np.ascontiguousarray(U[:, h0:h0 + h_sh].T)
        in_maps.append(dict(wct=wct, ctxc=ctxc, ws=ws, ut=ut, sbt=sbt, emat=emat))
    return in_maps


def kernel(context, solution, W, U, b, c):
    context = np.ascontiguousarray(np.asarray(context, np.float32))
    solution = np.ascontiguousarray(np.asarray(solution, np.float32))
    W = np.ascontiguousarray(np.asarray(W, np.float32))
    U = np.ascontiguousarray(np.asarray(U, np.float32))
    b = np.ascontiguousarray(np.asarray(b, np.float32))
    c = np.ascontiguousarray(np.asarray(c, np.float32))

    nc = build_core_kernel()
    in_maps = make_in_maps(context, solution, W, U, c)
    res = run_bass_kernel_spmd(nc, in_maps, core_ids=list(range(N_CORES)),
                               trace=TRACE)
    global LAST_RESULT
    LAST_RESULT = res

    partial = np.zeros(DIM_SOL, np.float32)
    for r in res.results:
        partial += r["outp"].reshape(DIM_SOL)

    logits = (b + partial).astype(np.float32)
    p_dist = (1.0 / (1.0 + np.exp(-logits, dtype=np.float32))).astype(np.float32)
    terms = (np.power(p_dist, solution) +
             np.power(np.float32(1.0) - p_dist[0],
                      np.float32(1.0) - solution)).astype(np.float32)
    p_val = np.prod(terms, dtype=np.float32)
    return (np.float32(p_val), p_dist)


# revision 14
# speedup vs baseline: 1.7936x; 1.7936x over previous
"""BinaryNADE Trainium2 kernel (8-core SPMD, h-sharded, d-on-partitions).

Math (reference):
    base = c + W_ctx @ context                      # [H]
    contrib = W_sol * s[None, :]                    # [H, D]
    A = base[:, None] + exclusive_cumsum_d(contrib) # [H, D]
    Hmat = sigmoid(A)                               # [H, D]
    logit[d] = b[d] + sum_h U[d, h] * Hmat[h, d]
    p_dist = sigmoid(logit)
    p_val = prod(p_dist**s + (1 - p_dist[0])**(1 - s))

Sharding: each of the 8 cores owns 512 rows of W (and the matching 512
columns of U); per-core partial dot products are summed on the host, which
also applies b, the final sigmoid, and the p_val reduction (O(D) work).

Per-core layout: d on partitions (64 tiles of 128), h along free (512).
The exclusive cumsum over d becomes per-tile strictly-triangular matmuls on
the TensorE plus a two-level carry: per-tile totals accumulate into one PSUM
bank via one-hot-column lhsT matrices, a single [65,64] triangular matmul
turns (base, totals) into per-tile offsets, and each tile's offset row is
broadcast across partitions with a rank-1 matmul into the same PSUM bank as
the triangular matmul. Data tensors travel in fp16 (PSUM accumulates fp32);
the host-side fp16 rounding contributes ~1e-4 relative error to p_dist.

Pipeline per d-tile:
    phase 1: contrib = wst * s[p]      (VectorE tensor_scalar, fp16 4x mode)
             totals[dt,:] += 1^T contrib  (TensorE, one-hot lhsT)
    phase 2: offs = scan_tri^T @ [base; totals]   (one TensorE matmul)
    phase 3: psum = tri^T @ contrib + 1 @ offs[dt]  (TensorE)
             Hm = sigmoid(psum)        (ScalarE, PSUM->SBUF fp16)
             out[:,dt] = sum_h Hm*ut   (VectorE tensor_tensor_reduce)
"""

import numpy as np

import concourse.bass as bass
import concourse.bacc as bacc
import concourse.mybir as mybir
from concourse.tile import TileContext
from concourse.bass_utils import run_bass_kernel_spmd

F32 = mybir.dt.float32
F16 = mybir.dt.float16

TRACE = False       # set by test harness to capture an NTFF profile
LAST_RESULT = None

DIM_SOL = 8192      # D
DIM_CONTEXT = 2048  # C
DIM_HIDDEN = 4096   # H
N_CORES = 8
H_SH = DIM_HIDDEN // N_CORES   # 512 hidden rows per core


def build_core_kernel(h_sh=H_SH, c_dim=DIM_CONTEXT, d_dim=DIM_SOL):
    """Per-core Bass program; all cores run it on their own shard."""
    assert h_sh % 512 == 0 or h_sh in (256, 512)
    assert d_dim % 128 == 0 and c_dim % 128 == 0
    dt_n = d_dim // 128           # number of d-tiles
    assert dt_n <= 64             # totals/offsets live on <=64 psum partitions
    base_row = 32 if dt_n <= 32 else 64   # allowed engine start partition
    kt_n = c_dim // 128 + 1       # base k-tiles incl. the c-vector row
    chunk_dt = min(16, dt_n)      # d-tiles per streamed DMA chunk
    n_chunks = dt_n // chunk_dt
    assert dt_n % chunk_dt == 0

    nc = bacc.Bacc("TRN2", target_bir_lowering=False, debug=False)

    # tiled [128, dt_n*h_sh] fp16: (p, dt*h_sh + h) = X[128*dt + p, h]
    wst = nc.dram_tensor("wst", [128, dt_n * h_sh], F16, kind="ExternalInput")
    utt = nc.dram_tensor("utt", [128, dt_n * h_sh], F16, kind="ExternalInput")
    wctt = nc.dram_tensor("wctt", [128, kt_n * h_sh], F16, kind="ExternalInput")
    ctxc = nc.dram_tensor("ctxc", [128, kt_n], F16, kind="ExternalInput")
    scol = nc.dram_tensor("scol", [128, dt_n], F32, kind="ExternalInput")
    emat = nc.dram_tensor("emat", [128, dt_n * dt_n], F16, kind="ExternalInput")
    trit = nc.dram_tensor("trit", [128, 128], F16, kind="ExternalInput")
    strib = nc.dram_tensor("strib", [base_row + 1, dt_n * 128], F16,
                           kind="ExternalInput")
    outp = nc.dram_tensor("outp", [128, dt_n], F32, kind="ExternalOutput")

    with TileContext(nc) as tc:
        with (
            tc.tile_pool(name="const", bufs=1) as constp,
            tc.tile_pool(name="wstp", bufs=2) as wstp,
            tc.tile_pool(name="uttp", bufs=2) as uttp,
            tc.tile_pool(name="hmp", bufs=3) as hmp,
            tc.tile_pool(name="vscrp", bufs=2) as vscrp,
            tc.tile_pool(name="psA", bufs=3, space="PSUM") as psap,
            tc.tile_pool(name="psmisc", bufs=1, space="PSUM") as psmp,
        ):
            # ---- constants -------------------------------------------------
            ctx_sb = constp.tile([128, kt_n], F16)
            nc.sync.dma_start(out=ctx_sb[:, :], in_=ctxc[:, :])
            scol_sb = constp.tile([128, dt_n], F32)
            nc.sync.dma_start(out=scol_sb[:, :], in_=scol[:, :])
            em_sb = constp.tile([128, dt_n * dt_n], F16)
            nc.sync.dma_start(out=em_sb[:, :], in_=emat[:, :])
            tri_sb = constp.tile([128, 128], F16)
            nc.sync.dma_start(out=tri_sb[:, :], in_=trit[:, :])
            strib_sb = constp.tile([base_row + 1, dt_n * 128], F16)
            nc.sync.dma_start(out=strib_sb[:, :], in_=strib[:, :])
            wct_sb = constp.tile([128, kt_n * h_sh], F16)
            nc.sync.dma_start(out=wct_sb[:, :], in_=wctt[:, :])

            contrib = constp.tile([128, dt_n * h_sh], F16)
            totals_sb = constp.tile([base_row + 1, h_sh], F16)
            nc.vector.memset(totals_sb[:, :], 0.0)
            out_sb = constp.tile([128, dt_n], F32)

            # ---- base row: [1, h_sh] = ctx^T @ W_ctx^T (+ c via aug row) ---
            ps_base = psmp.tile([1, h_sh], F32)
            for kt in range(kt_n):
                nc.tensor.matmul(
                    ps_base[:, :],
                    ctx_sb[:, kt:kt + 1],
                    wct_sb[:, h_sh * kt:h_sh * (kt + 1)],
                    start=(kt == 0), stop=(kt == kt_n - 1),
                )
            nc.scalar.copy(totals_sb[base_row:base_row + 1, :], ps_base[:, :])

            # ---- phase 1: contrib tiles + per-tile totals ------------------
            ps_tot = psmp.tile([dt_n, h_sh], F32)
            for ch in range(n_chunks):
                wst_t = wstp.tile([128, chunk_dt * h_sh], F16)
                nc.sync.dma_start(
                    out=wst_t[:, :],
                    in_=wst[:, chunk_dt * h_sh * ch:chunk_dt * h_sh * (ch + 1)],
                )
                for j in range(chunk_dt):
                    dt = ch * chunk_dt + j
                    nc.vector.tensor_scalar_mul(
                        contrib[:, h_sh * dt:h_sh * (dt + 1)],
                        wst_t[:, h_sh * j:h_sh * (j + 1)],
                        scol_sb[:, dt:dt + 1],
                    )
                    nc.tensor.matmul(
                        ps_tot[:, :],
                        em_sb[:, dt_n * dt:dt_n * (dt + 1)],
                        contrib[:, h_sh * dt:h_sh * (dt + 1)],
                        start=(dt == 0), stop=(dt == dt_n - 1),
                    )
            nc.scalar.copy(totals_sb[0:dt_n, :], ps_tot[:, :])

            # ---- phase 3: A tiles, sigmoid, fused dot ----------------------
            for ch in range(n_chunks):
                utt_t = uttp.tile([128, chunk_dt * h_sh], F16)
                nc.sync.dma_start(
                    out=utt_t[:, :],
                    in_=utt[:, chunk_dt * h_sh * ch:chunk_dt * h_sh * (ch + 1)],
                )
                for j in range(chunk_dt):
                    dt = ch * chunk_dt + j
                    ps_a = psap.tile([128, h_sh], F32)
                    nc.tensor.matmul(ps_a[:, :], tri_sb[:, :],
                                     contrib[:, h_sh * dt:h_sh * (dt + 1)],
                                     start=True, stop=False)
                    nc.tensor.matmul(ps_a[:, :],
                                     strib_sb[:, 128 * dt:128 * (dt + 1)],
                                     totals_sb[:, :],
                                     start=False, stop=True)
                    hm_t = hmp.tile([128, h_sh], F16)
                    nc.scalar.activation(hm_t[:, :], ps_a[:, :],
                                         mybir.ActivationFunctionType.Sigmoid)
                    vscr = vscrp.tile([128, h_sh], F16)
                    nc.vector.tensor_mul(vscr[:, :], hm_t[:, :],
                                         utt_t[:, h_sh * j:h_sh * (j + 1)])
                    vred = vscrp.tile([128, h_sh], F16, name=f"vred", tag="vred")
                    nc.scalar.activation(
                        vred[:, :], vscr[:, :],
                        mybir.ActivationFunctionType.Copy,
                        accum_out=out_sb[:, dt:dt + 1])

            nc.sync.dma_start(out=outp[:, :], in_=out_sb[:, :])

    nc.compile()
    return nc


def make_in_maps(context, solution, W, U, c,
                 h_sh=H_SH, c_dim=DIM_CONTEXT, d_dim=DIM_SOL, n_cores=N_CORES):
    """Host-side shard/layout prep. Layout + dtype only — no model math."""
    f16 = np.float16
    dt_n = d_dim // 128
    kt_n = c_dim // 128 + 1

    ctxa = np.zeros(kt_n * 128, np.float32)
    ctxa[:c_dim] = context
    ctxa[c_dim] = 1.0
    ctxc = np.ascontiguousarray(ctxa.reshape(kt_n, 128).T).astype(f16)

    scol = np.ascontiguousarray(
        solution.reshape(dt_n, 128).T).astype(np.float32)

    emat = np.zeros((128, dt_n * dt_n), f16)
    for dt in range(dt_n):
        emat[:, dt_n * dt + dt] = 1.0

    trit = np.triu(np.ones((128, 128), f16), 1)          # [p, i] = 1 if p < i
    # strib[p, dt*128 + i] = 1 if p < dt (totals rows, strict prefix) or
    # p == base_row (base always included). Offset-scan folded into the
    # per-tile partition-broadcast matmul.
    base_row = 32 if dt_n <= 32 else 64
    strib = np.zeros((base_row + 1, dt_n * 128), f16)
    for dt in range(dt_n):
        strib[:dt, 128 * dt:128 * (dt + 1)] = 1.0
    strib[base_row, :] = 1.0

    def tile_pd(x):  # [d_dim, h_sh] -> [128, dt_n*h_sh] fp16 tiled layout
        return np.ascontiguousarray(
            x.reshape(dt_n, 128, h_sh).transpose(1, 0, 2).reshape(
                128, dt_n * h_sh)).astype(f16)

    in_maps = []
    for core in range(n_cores):
        h0 = core * h_sh
        wst = tile_pd(np.ascontiguousarray(W[h0:h0 + h_sh, c_dim:].T))
        utt = tile_pd(np.ascontiguousarray(U[:, h0:h0 + h_sh]))
        wcta = np.zeros((kt_n * 128, h_sh), np.float32)
        wcta[:c_dim] = W[h0:h0 + h_sh, :c_dim].T
        wcta[c_dim] = c[h0:h0 + h_sh]
        wctt = np.ascontiguousarray(
            wcta.reshape(kt_n, 128, h_sh).transpose(1, 0, 2).reshape(
                128, kt_n * h_sh)).astype(f16)
        in_maps.append(dict(wst=wst, utt=utt, wctt=wctt, ctxc=ctxc,
                            scol=scol, emat=emat, trit=trit, strib=strib))
    return in_maps


def kernel(context, solution, W, U, b, c):
    context = np.ascontiguousarray(np.asarray(context, np.float32))
    solution = np.ascontiguousarray(np.asarray(solution, np.float32))
    W = np.ascontiguousarray(np.asarray(W, np.float32))
    U = np.ascontiguousarray(np.asarray(U, np.float32))
    b = np.ascontiguousarray(np.asarray(b, np.float32))
    c = np.ascontiguousarray(np.asarray(c, np.float32))

    nc = build_core_kernel()
    in_maps = make_in_maps(context, solution, W, U, c)
    res = run_bass_kernel_spmd(nc, in_maps, core_ids=list(range(N_CORES)),
                               trace=TRACE)
    global LAST_RESULT
    LAST_RESULT = res

    dt_n = DIM_SOL // 128
    partial = np.zeros(DIM_SOL, np.float32)
    for r in res.results:
        partial += r["outp"].T.reshape(DIM_SOL)  # d = 128*dt + p

    logits = (b + partial).astype(np.float32)
    p_dist = (1.0 / (1.0 + np.exp(-logits, dtype=np.float32))).astype(np.float32)
    terms = (np.power(p_dist, solution) +
             np.power(np.float32(1.0) - p_dist[0],
                      np.float32(1.0) - solution)).astype(np.float32)
    p_val = np.prod(terms, dtype=np.float32)
    return (np.float32(p_val), p_dist)


# revision 15
# speedup vs baseline: 2.6035x; 1.4516x over previous
"""BinaryNADE Trainium2 kernel (8-core SPMD, h-sharded, d-on-partitions).

Math (reference):
    base = c + W_ctx @ context                      # [H]
    contrib = W_sol * s[None, :]                    # [H, D]
    A = base[:, None] + exclusive_cumsum_d(contrib) # [H, D]
    Hmat = sigmoid(A)                               # [H, D]
    logit[d] = b[d] + sum_h U[d, h] * Hmat[h, d]
    p_dist = sigmoid(logit)
    p_val = prod(p_dist**s + (1 - p_dist[0])**(1 - s))

Sharding: each of the 8 cores owns 512 rows of W (and the matching 512
columns of U); per-core partial dot products are summed on the host, which
also applies b, the final sigmoid, and the p_val reduction (O(D) work).

Per-core layout: d on partitions (64 tiles of 128), h along free (512).
The exclusive cumsum over d becomes per-tile strictly-triangular matmuls on
the TensorE plus a two-level carry: per-tile totals accumulate into one PSUM
bank via one-hot-column lhsT matrices, a single [65,64] triangular matmul
turns (base, totals) into per-tile offsets, and each tile's offset row is
broadcast across partitions with a rank-1 matmul into the same PSUM bank as
the triangular matmul. Data tensors travel in fp16 (PSUM accumulates fp32);
the host-side fp16 rounding contributes ~1e-4 relative error to p_dist.

Pipeline per d-tile:
    phase 1: contrib = wst * s[p]      (VectorE tensor_scalar, fp16 4x mode)
             totals[dt,:] += 1^T contrib  (TensorE, one-hot lhsT)
    phase 2: offs = scan_tri^T @ [base; totals]   (one TensorE matmul)
    phase 3: psum = tri^T @ contrib + 1 @ offs[dt]  (TensorE)
             Hm = sigmoid(psum)        (ScalarE, PSUM->SBUF fp16)
             out[:,dt] = sum_h Hm*ut   (VectorE tensor_tensor_reduce)
"""

import numpy as np

import concourse.bass as bass
import concourse.bacc as bacc
import concourse.mybir as mybir
from concourse.tile import TileContext
from concourse.bass_utils import run_bass_kernel_spmd

F32 = mybir.dt.float32
F16 = mybir.dt.float16

TRACE = False       # set by test harness to capture an NTFF profile
LAST_RESULT = None

DIM_SOL = 8192      # D
DIM_CONTEXT = 2048  # C
DIM_HIDDEN = 4096   # H
N_CORES = 8
H_SH = DIM_HIDDEN // N_CORES   # 512 hidden rows per core


def build_core_kernel(h_sh=H_SH, c_dim=DIM_CONTEXT, d_dim=DIM_SOL):
    """Per-core Bass program; all cores run it on their own shard."""
    assert h_sh % 512 == 0 or h_sh in (256, 512)
    assert d_dim % 128 == 0 and c_dim % 128 == 0
    dt_n = d_dim // 128           # number of d-tiles
    assert dt_n <= 64             # totals/offsets live on <=64 psum partitions
    base_row = 32 if dt_n <= 32 else 64   # allowed engine start partition
    kt_n = c_dim // 128 + 1       # base k-tiles incl. the c-vector row
    chunk_dt = min(16, dt_n)      # d-tiles per streamed DMA chunk
    n_chunks = dt_n // chunk_dt
    assert dt_n % chunk_dt == 0

    nc = bacc.Bacc("TRN2", target_bir_lowering=False, debug=False)

    # tiled [128, dt_n*h_sh] fp16: (p, dt*h_sh + h) = X[128*dt + p, h]
    wst = nc.dram_tensor("wst", [128, dt_n * h_sh], F16, kind="ExternalInput")
    utt = nc.dram_tensor("utt", [128, dt_n * h_sh], F16, kind="ExternalInput")
    wctt = nc.dram_tensor("wctt", [128, kt_n * h_sh], F16, kind="ExternalInput")
    ctxc = nc.dram_tensor("ctxc", [128, kt_n], F16, kind="ExternalInput")
    scol = nc.dram_tensor("scol", [128, dt_n], F32, kind="ExternalInput")
    emat = nc.dram_tensor("emat", [128, dt_n * dt_n], F16, kind="ExternalInput")
    trit = nc.dram_tensor("trit", [128, 128], F16, kind="ExternalInput")
    strib = nc.dram_tensor("strib", [base_row + 1, dt_n * 128], F16,
                           kind="ExternalInput")
    outp = nc.dram_tensor("outp", [128, dt_n], F32, kind="ExternalOutput")

    with TileContext(nc) as tc:
        with (
            tc.tile_pool(name="const", bufs=1) as constp,
            tc.tile_pool(name="wstp", bufs=2) as wstp,
            tc.tile_pool(name="uttp", bufs=2) as uttp,
            tc.tile_pool(name="hmp", bufs=4) as hmp,
            tc.tile_pool(name="vscrp", bufs=3) as vscrp,
            tc.tile_pool(name="psA", bufs=4, space="PSUM") as psap,
            tc.tile_pool(name="psmisc", bufs=1, space="PSUM") as psmp,
        ):
            # ---- constants -------------------------------------------------
            ctx_sb = constp.tile([128, kt_n], F16)
            nc.sync.dma_start(out=ctx_sb[:, :], in_=ctxc[:, :])
            scol_sb = constp.tile([128, dt_n], F32)
            nc.sync.dma_start(out=scol_sb[:, :], in_=scol[:, :])
            em_sb = constp.tile([128, dt_n * dt_n], F16)
            nc.sync.dma_start(out=em_sb[:, :], in_=emat[:, :])
            tri_sb = constp.tile([128, 128], F16)
            nc.sync.dma_start(out=tri_sb[:, :], in_=trit[:, :])
            strib_sb = constp.tile([base_row + 1, dt_n * 128], F16)
            nc.sync.dma_start(out=strib_sb[:, :], in_=strib[:, :])
            wct_sb = constp.tile([128, kt_n * h_sh], F16)
            nc.sync.dma_start(out=wct_sb[:, :], in_=wctt[:, :])

            contrib = constp.tile([128, dt_n * h_sh], F16)
            totals_sb = constp.tile([base_row + 1, h_sh], F16)
            nc.vector.memset(totals_sb[:, :], 0.0)
            out_sb = constp.tile([128, dt_n], F32)

            # ---- base row: [1, h_sh] = ctx^T @ W_ctx^T (+ c via aug row) ---
            ps_base = psmp.tile([1, h_sh], F32)
            for kt in range(kt_n):
                nc.tensor.matmul(
                    ps_base[:, :],
                    ctx_sb[:, kt:kt + 1],
                    wct_sb[:, h_sh * kt:h_sh * (kt + 1)],
                    start=(kt == 0), stop=(kt == kt_n - 1),
                )
            nc.scalar.copy(totals_sb[base_row:base_row + 1, :], ps_base[:, :])

            # ---- phase 1: contrib tiles + per-tile totals ------------------
            ps_tot = psmp.tile([dt_n, h_sh], F32)
            for ch in range(n_chunks):
                wst_t = wstp.tile([128, chunk_dt * h_sh], F16)
                nc.sync.dma_start(
                    out=wst_t[:, :],
                    in_=wst[:, chunk_dt * h_sh * ch:chunk_dt * h_sh * (ch + 1)],
                )
                for j in range(chunk_dt):
                    dt = ch * chunk_dt + j
                    nc.vector.tensor_scalar_mul(
                        contrib[:, h_sh * dt:h_sh * (dt + 1)],
                        wst_t[:, h_sh * j:h_sh * (j + 1)],
                        scol_sb[:, dt:dt + 1],
                    )
                    nc.tensor.matmul(
                        ps_tot[:, :],
                        em_sb[:, dt_n * dt:dt_n * (dt + 1)],
                        contrib[:, h_sh * dt:h_sh * (dt + 1)],
                        start=(dt == 0), stop=(dt == dt_n - 1),
                    )
            nc.scalar.copy(totals_sb[0:dt_n, :], ps_tot[:, :])

            # ---- phase 3: A tiles, sigmoid, fused dot ----------------------
            for ch in range(n_chunks):
                utt_t = uttp.tile([128, chunk_dt * h_sh], F16)
                nc.sync.dma_start(
                    out=utt_t[:, :],
                    in_=utt[:, chunk_dt * h_sh * ch:chunk_dt * h_sh * (ch + 1)],
                )
                for j in range(chunk_dt):
                    dt = ch * chunk_dt + j
                    ps_a = psap.tile([128, h_sh], F32)
                    nc.tensor.matmul(ps_a[:, :], tri_sb[:, :],
                                     contrib[:, h_sh * dt:h_sh * (dt + 1)],
                                     start=True, stop=False)
                    nc.tensor.matmul(ps_a[:, :],
                                     strib_sb[:, 128 * dt:128 * (dt + 1)],
                                     totals_sb[:, :],
                                     start=False, stop=True)
                    hm_t = hmp.tile([128, h_sh], F16)
                    nc.scalar.activation(hm_t[:, :], ps_a[:, :],
                                         mybir.ActivationFunctionType.Sigmoid)
                    vscr = vscrp.tile([128, h_sh], F16)
                    nc.vector.scalar_tensor_tensor(
                        vscr[:, :], hm_t[:, :], 1.0,
                        utt_t[:, h_sh * j:h_sh * (j + 1)],
                        mybir.AluOpType.mult, mybir.AluOpType.mult,
                        accum_out=out_sb[:, dt:dt + 1])

            nc.sync.dma_start(out=outp[:, :], in_=out_sb[:, :])

    nc.compile()
    return nc


def make_in_maps(context, solution, W, U, c,
                 h_sh=H_SH, c_dim=DIM_CONTEXT, d_dim=DIM_SOL, n_cores=N_CORES):
    """Host-side shard/layout prep. Layout + dtype only — no model math."""
    f16 = np.float16
    dt_n = d_dim // 128
    kt_n = c_dim // 128 + 1

    ctxa = np.zeros(kt_n * 128, np.float32)
    ctxa[:c_dim] = context
    ctxa[c_dim] = 1.0
    ctxc = np.ascontiguousarray(ctxa.reshape(kt_n, 128).T).astype(f16)

    scol = np.ascontiguousarray(
        solution.reshape(dt_n, 128).T).astype(np.float32)

    emat = np.zeros((128, dt_n * dt_n), f16)
    for dt in range(dt_n):
        emat[:, dt_n * dt + dt] = 1.0

    trit = np.triu(np.ones((128, 128), f16), 1)          # [p, i] = 1 if p < i
    # strib[p, dt*128 + i] = 1 if p < dt (totals rows, strict prefix) or
    # p == base_row (base always included). Offset-scan folded into the
    # per-tile partition-broadcast matmul.
    base_row = 32 if dt_n <= 32 else 64
    strib = np.zeros((base_row + 1, dt_n * 128), f16)
    for dt in range(dt_n):
        strib[:dt, 128 * dt:128 * (dt + 1)] = 1.0
    strib[base_row, :] = 1.0

    def tile_pd(x):  # [d_dim, h_sh] -> [128, dt_n*h_sh] fp16 tiled layout
        return np.ascontiguousarray(
            x.reshape(dt_n, 128, h_sh).transpose(1, 0, 2).reshape(
                128, dt_n * h_sh)).astype(f16)

    in_maps = []
    for core in range(n_cores):
        h0 = core * h_sh
        wst = tile_pd(np.ascontiguousarray(W[h0:h0 + h_sh, c_dim:].T))
        utt = tile_pd(np.ascontiguousarray(U[:, h0:h0 + h_sh]))
        wcta = np.zeros((kt_n * 128, h_sh), np.float32)
        wcta[:c_dim] = W[h0:h0 + h_sh, :c_dim].T
        wcta[c_dim] = c[h0:h0 + h_sh]
        wctt = np.ascontiguousarray(
            wcta.reshape(kt_n, 128, h_sh).transpose(1, 0, 2).reshape(
                128, kt_n * h_sh)).astype(f16)
        in_maps.append(dict(wst=wst, utt=utt, wctt=wctt, ctxc=ctxc,
                            scol=scol, emat=emat, trit=trit, strib=strib))
    return in_maps


def kernel(context, solution, W, U, b, c):
    context = np.ascontiguousarray(np.asarray(context, np.float32))
    solution = np.ascontiguousarray(np.asarray(solution, np.float32))
    W = np.ascontiguousarray(np.asarray(W, np.float32))
    U = np.ascontiguousarray(np.asarray(U, np.float32))
    b = np.ascontiguousarray(np.asarray(b, np.float32))
    c = np.ascontiguousarray(np.asarray(c, np.float32))

    nc = build_core_kernel()
    in_maps = make_in_maps(context, solution, W, U, c)
    res = run_bass_kernel_spmd(nc, in_maps, core_ids=list(range(N_CORES)),
                               trace=TRACE)
    global LAST_RESULT
    LAST_RESULT = res

    dt_n = DIM_SOL // 128
    partial = np.zeros(DIM_SOL, np.float32)
    for r in res.results:
        partial += r["outp"].T.reshape(DIM_SOL)  # d = 128*dt + p

    logits = (b + partial).astype(np.float32)
    p_dist = (1.0 / (1.0 + np.exp(-logits, dtype=np.float32))).astype(np.float32)
    terms = (np.power(p_dist, solution) +
             np.power(np.float32(1.0) - p_dist[0],
                      np.float32(1.0) - solution)).astype(np.float32)
    p_val = np.prod(terms, dtype=np.float32)
    return (np.float32(p_val), p_dist)


# revision 17
# speedup vs baseline: 2.7070x; 1.0397x over previous
"""BinaryNADE Trainium2 kernel (8-core SPMD, h-sharded, d-on-partitions).

Math (reference):
    base = c + W_ctx @ context                      # [H]
    contrib = W_sol * s[None, :]                    # [H, D]
    A = base[:, None] + exclusive_cumsum_d(contrib) # [H, D]
    Hmat = sigmoid(A)                               # [H, D]
    logit[d] = b[d] + sum_h U[d, h] * Hmat[h, d]
    p_dist = sigmoid(logit)
    p_val = prod(p_dist**s + (1 - p_dist[0])**(1 - s))

Sharding: each of the 8 cores owns 512 rows of W (and the matching 512
columns of U); per-core partial dot products are summed on the host, which
also applies b, the final sigmoid, and the p_val reduction (O(D) work).

Per-core layout: d on partitions (64 tiles of 128), h along free (512).
The exclusive cumsum over d becomes per-tile strictly-triangular matmuls on
the TensorE plus a two-level carry: per-tile totals accumulate into one PSUM
bank via one-hot-column lhsT matrices, a single [65,64] triangular matmul
turns (base, totals) into per-tile offsets, and each tile's offset row is
broadcast across partitions with a rank-1 matmul into the same PSUM bank as
the triangular matmul. Data tensors travel in fp16 (PSUM accumulates fp32);
the host-side fp16 rounding contributes ~1e-4 relative error to p_dist.

Pipeline per d-tile:
    phase 1: contrib = wst * s[p]      (VectorE tensor_scalar, fp16 4x mode)
             totals[dt,:] += 1^T contrib  (TensorE, one-hot lhsT)
    phase 2: offs = scan_tri^T @ [base; totals]   (one TensorE matmul)
    phase 3: psum = tri^T @ contrib + 1 @ offs[dt]  (TensorE)
             Hm = sigmoid(psum)        (ScalarE, PSUM->SBUF fp16)
             out[:,dt] = sum_h Hm*ut   (VectorE tensor_tensor_reduce)
"""

import numpy as np

import concourse.bass as bass
import concourse.bacc as bacc
import concourse.mybir as mybir
from concourse.tile import TileContext
from concourse.bass_utils import run_bass_kernel_spmd

F32 = mybir.dt.float32
F16 = mybir.dt.float16

TRACE = False       # set by test harness to capture an NTFF profile
LAST_RESULT = None

DIM_SOL = 8192      # D
DIM_CONTEXT = 2048  # C
DIM_HIDDEN = 4096   # H
N_CORES = 8
H_SH = DIM_HIDDEN // N_CORES   # 512 hidden rows per core


def build_core_kernel(h_sh=H_SH, c_dim=DIM_CONTEXT, d_dim=DIM_SOL):
    """Per-core Bass program; all cores run it on their own shard."""
    assert h_sh % 512 == 0 or h_sh in (256, 512)
    assert d_dim % 128 == 0 and c_dim % 128 == 0
    dt_n = d_dim // 128           # number of d-tiles
    assert dt_n <= 64             # totals/offsets live on <=64 psum partitions
    base_row = 32 if dt_n <= 32 else 64   # allowed engine start partition
    kt_n = c_dim // 128 + 1       # base k-tiles incl. the c-vector row
    chunk_dt = min(16, dt_n)      # d-tiles per streamed DMA chunk
    n_chunks = dt_n // chunk_dt
    assert dt_n % chunk_dt == 0

    nc = bacc.Bacc("TRN2", target_bir_lowering=False, debug=False)

    # tiled [128, dt_n*h_sh] fp16: (p, dt*h_sh + h) = X[128*dt + p, h]
    wst = nc.dram_tensor("wst", [128, dt_n * h_sh], F16, kind="ExternalInput")
    utt = nc.dram_tensor("utt", [128, dt_n * h_sh], F16, kind="ExternalInput")
    wctt = nc.dram_tensor("wctt", [128, kt_n * h_sh], F16, kind="ExternalInput")
    ctxc = nc.dram_tensor("ctxc", [128, kt_n], F16, kind="ExternalInput")
    scol = nc.dram_tensor("scol", [128, dt_n], F32, kind="ExternalInput")
    emat = nc.dram_tensor("emat", [128, dt_n * dt_n], F16, kind="ExternalInput")
    trit = nc.dram_tensor("trit", [128, 128], F16, kind="ExternalInput")
    strib = nc.dram_tensor("strib", [base_row + 1, dt_n * 128], F16,
                           kind="ExternalInput")
    outp = nc.dram_tensor("outp", [128, dt_n], F32, kind="ExternalOutput")

    with TileContext(nc) as tc:
        with (
            tc.tile_pool(name="const", bufs=1) as constp,
            tc.tile_pool(name="wstp", bufs=2) as wstp,
            tc.tile_pool(name="uttp", bufs=2) as uttp,
            tc.tile_pool(name="hmp", bufs=4) as hmp,
            tc.tile_pool(name="vscrp", bufs=3) as vscrp,
            tc.tile_pool(name="psA", bufs=1, space="PSUM") as psap,
            tc.tile_pool(name="psmisc", bufs=1, space="PSUM") as psmp,
        ):
            # ---- constants -------------------------------------------------
            scol_sb = constp.tile([128, dt_n], F32)
            nc.sync.dma_start(out=scol_sb[:, :], in_=scol[:, :])
            em_sb = constp.tile([128, dt_n * dt_n], F16)
            nc.sync.dma_start(out=em_sb[:, :], in_=emat[:, :])
            ctx_sb = constp.tile([128, kt_n], F16)
            nc.sync.dma_start(out=ctx_sb[:, :], in_=ctxc[:, :])
            tri_sb = constp.tile([128, 128], F16)
            nc.sync.dma_start(out=tri_sb[:, :], in_=trit[:, :])
            wct_sb = constp.tile([128, kt_n * h_sh], F16)
            nc.sync.dma_start(out=wct_sb[:, :], in_=wctt[:, :])
            strib_sb = constp.tile([base_row + 1, dt_n * 128], F16)
            nc.sync.dma_start(out=strib_sb[:, :], in_=strib[:, :])

            contrib = constp.tile([128, dt_n * h_sh], F16)
            totals_sb = constp.tile([base_row + 1, h_sh], F16)
            nc.vector.memset(totals_sb[:, :], 0.0)
            out_sb = constp.tile([128, dt_n], F32)

            # ---- base row: [1, h_sh] = ctx^T @ W_ctx^T (+ c via aug row) ---
            ps_base = psmp.tile([1, h_sh], F32)
            for kt in range(kt_n):
                nc.tensor.matmul(
                    ps_base[:, :],
                    ctx_sb[:, kt:kt + 1],
                    wct_sb[:, h_sh * kt:h_sh * (kt + 1)],
                    start=(kt == 0), stop=(kt == kt_n - 1),
                )
            nc.scalar.copy(totals_sb[base_row:base_row + 1, :], ps_base[:, :])

            # ---- phase 1: contrib tiles + per-tile totals ------------------
            ps_tot = psmp.tile([dt_n, h_sh], F32)
            for ch in range(n_chunks):
                wst_t = wstp.tile([128, chunk_dt * h_sh], F16)
                nc.sync.dma_start(
                    out=wst_t[:, :],
                    in_=wst[:, chunk_dt * h_sh * ch:chunk_dt * h_sh * (ch + 1)],
                )
                for j in range(chunk_dt):
                    dt = ch * chunk_dt + j
                    nc.vector.tensor_scalar_mul(
                        contrib[:, h_sh * dt:h_sh * (dt + 1)],
                        wst_t[:, h_sh * j:h_sh * (j + 1)],
                        scol_sb[:, dt:dt + 1],
                    )
                    nc.tensor.matmul(
                        ps_tot[:, :],
                        em_sb[:, dt_n * dt:dt_n * (dt + 1)],
                        contrib[:, h_sh * dt:h_sh * (dt + 1)],
                        start=(dt == 0), stop=(dt == dt_n - 1),
                    )
            nc.scalar.copy(totals_sb[0:dt_n, :], ps_tot[:, :])

            # ---- phase 3: A tiles, sigmoid, fused dot ----------------------
            for ch in range(n_chunks):
                utt_t = uttp.tile([128, chunk_dt * h_sh], F16)
                nc.sync.dma_start(
                    out=utt_t[:, :],
                    in_=utt[:, chunk_dt * h_sh * ch:chunk_dt * h_sh * (ch + 1)],
                )
                for j0 in range(0, chunk_dt, 4):
                    grp = range(j0, min(j0 + 4, chunk_dt))
                    ps_as = {}
                    for j in grp:
                        dt = ch * chunk_dt + j
                        ps_a = psap.tile([128, h_sh], F32, name=f"ps_a{j % 4}",
                                         tag=f"ps_a{j % 4}")
                        ps_as[j] = ps_a
                        nc.tensor.matmul(ps_a[:, :], tri_sb[:, :],
                                         contrib[:, h_sh * dt:h_sh * (dt + 1)],
                                         start=True, stop=False)
                    for j in grp:
                        dt = ch * chunk_dt + j
                        nc.tensor.matmul(ps_as[j][:, :],
                                         strib_sb[:, 128 * dt:128 * (dt + 1)],
                                         totals_sb[:, :],
                                         start=False, stop=True)
                    for j in grp:
                        dt = ch * chunk_dt + j
                        hm_t = hmp.tile([128, h_sh], F16)
                        nc.scalar.activation(hm_t[:, :], ps_as[j][:, :],
                                             mybir.ActivationFunctionType.Sigmoid)
                        vscr = vscrp.tile([128, h_sh], F16)
                        nc.vector.scalar_tensor_tensor(
                            vscr[:, :], hm_t[:, :], 1.0,
                            utt_t[:, h_sh * j:h_sh * (j + 1)],
                            mybir.AluOpType.mult, mybir.AluOpType.mult,
                            accum_out=out_sb[:, dt:dt + 1])

            nc.sync.dma_start(out=outp[:, :], in_=out_sb[:, :])

    nc.compile()
    return nc


def make_in_maps(context, solution, W, U, c,
                 h_sh=H_SH, c_dim=DIM_CONTEXT, d_dim=DIM_SOL, n_cores=N_CORES):
    """Host-side shard/layout prep. Layout + dtype only — no model math."""
    f16 = np.float16
    dt_n = d_dim // 128
    kt_n = c_dim // 128 + 1

    ctxa = np.zeros(kt_n * 128, np.float32)
    ctxa[:c_dim] = context
    ctxa[c_dim] = 1.0
    ctxc = np.ascontiguousarray(ctxa.reshape(kt_n, 128).T).astype(f16)

    scol = np.ascontiguousarray(
        solution.reshape(dt_n, 128).T).astype(np.float32)

    emat = np.zeros((128, dt_n * dt_n), f16)
    for dt in range(dt_n):
        emat[:, dt_n * dt + dt] = 1.0

    trit = np.triu(np.ones((128, 128), f16), 1)          # [p, i] = 1 if p < i
    # strib[p, dt*128 + i] = 1 if p < dt (totals rows, strict prefix) or
    # p == base_row (base always included). Offset-scan folded into the
    # per-tile partition-broadcast matmul.
    base_row = 32 if dt_n <= 32 else 64
    strib = np.zeros((base_row + 1, dt_n * 128), f16)
    for dt in range(dt_n):
        strib[:dt, 128 * dt:128 * (dt + 1)] = 1.0
    strib[base_row, :] = 1.0

    def tile_pd(x):  # [d_dim, h_sh] -> [128, dt_n*h_sh] fp16 tiled layout
        return np.ascontiguousarray(
            x.reshape(dt_n, 128, h_sh).transpose(1, 0, 2).reshape(
                128, dt_n * h_sh)).astype(f16)

    in_maps = []
    for core in range(n_cores):
        h0 = core * h_sh
        wst = tile_pd(np.ascontiguousarray(W[h0:h0 + h_sh, c_dim:].T))
        utt = tile_pd(np.ascontiguousarray(U[:, h0:h0 + h_sh]))
        wcta = np.zeros((kt_n * 128, h_sh), np.float32)
        wcta[:c_dim] = W[h0:h0 + h_sh, :c_dim].T
        wcta[c_dim] = c[h0:h0 + h_sh]
        wctt = np.ascontiguousarray(
            wcta.reshape(kt_n, 128, h_sh).transpose(1, 0, 2).reshape(
                128, kt_n * h_sh)).astype(f16)
        in_maps.append(dict(wst=wst, utt=utt, wctt=wctt, ctxc=ctxc,
                            scol=scol, emat=emat, trit=trit, strib=strib))
    return in_maps


def kernel(context, solution, W, U, b, c):
    context = np.ascontiguousarray(np.asarray(context, np.float32))
    solution = np.ascontiguousarray(np.asarray(solution, np.float32))
    W = np.ascontiguousarray(np.asarray(W, np.float32))
    U = np.ascontiguousarray(np.asarray(U, np.float32))
    b = np.ascontiguousarray(np.asarray(b, np.float32))
    c = np.ascontiguousarray(np.asarray(c, np.float32))

    nc = build_core_kernel()
    in_maps = make_in_maps(context, solution, W, U, c)
    res = run_bass_kernel_spmd(nc, in_maps, core_ids=list(range(N_CORES)),
                               trace=TRACE)
    global LAST_RESULT
    LAST_RESULT = res

    dt_n = DIM_SOL // 128
    partial = np.zeros(DIM_SOL, np.float32)
    for r in res.results:
        partial += r["outp"].T.reshape(DIM_SOL)  # d = 128*dt + p

    logits = (b + partial).astype(np.float32)
    p_dist = (1.0 / (1.0 + np.exp(-logits, dtype=np.float32))).astype(np.float32)
    terms = (np.power(p_dist, solution) +
             np.power(np.float32(1.0) - p_dist[0],
                      np.float32(1.0) - solution)).astype(np.float32)
    p_val = np.prod(terms, dtype=np.float32)
    return (np.float32(p_val), p_dist)
